# revision 9
# baseline (speedup 1.0000x reference)
# Trainium2 Bass kernel for CoAttentionModule (axial co-attention, 8 heads).
#
# Sharding: data-parallel over (direction, batch) = 2 x 4 = 8 NeuronCores.
# Core c computes weighted = _coattention(qf, rf)[b].T for its (d, b) pair;
# the host concatenates [features, weighted] per direction.
#
# On-chip layout: the hw axis is w-major everywhere (col = w*32 + i, i = h
# index); the host pre-permutes features and un-permutes the output. This
# makes every matmul stationary operand a contiguous SBUF slice (walrus
# requires single-free-dim weight APs).
#
# Precision plan (rel err ~1.7e-2 vs the 2e-2 gate, numpy-validated):
#   Q/K projections: single-pass fp8 DoubleRow (e4m3 weights AND e4m3
#     activations, both pre-scaled on host: x*16, W*1024 so weight values
#     clear the e4m3 subnormal region). The resulting q/k are stored bf16 at
#     16384x their true scale; the 16384^-2 is folded into the softmax exp
#     scale, and rel-embedding constants are host-scaled by 16384 to match.
#   V / O projections: plain bf16 (exact to ~2e-3), attention output bf16.
# This halves the Q/K matmul count vs hi+lo and keeps V/O at the bf16
# streaming floor, while the error budget stays comfortably under the gate.
#
# Per-core pipeline (fp32 PSUM accumulation everywhere):
#   qT = Wq8.T @ xq8 (+bq*S)      [c_out, hw]  fp8 DR single pass
#   kT = Wk8.T @ xr8  + RWF*S     RWF[c,(w,k)] = rel_emb[(k-w)%63, c]  (rel_w
#                                 folded into keys; bk cancels in softmax)
#   v  = xrb.T @ Wvb              [(w,k), c]   bf16, x-stationary
#   QAUG[t', col(w,i)] = sum_c relx[(t'-i)%63, c] q[c, col]  (only rows
#                                 t'<32 matter: kaug one-hot needs t'==k<32)
#   scores tile (head n, w-group of 4) [128=(w,i), 128=(w,k)]:
#       q.k' + QAUG.KAUG(one-hot) + WIND.KMASK(-1e30 off-diag mask channels)
#   softmax: exp(scale=1/(16*16384^2)) with accum_out row sums -> reciprocal
#   probsT via DVE 32x32 stream transpose (block-diagonal => exact transpose)
#   avT[c,(w,i)] = v.T @ probsT (bf16); outT = Wob.T @ attT + bo'  bf16
#   (bv folded on host: bo' = bv @ Wo + bo; bk dropped: softmax-invariant)
import numpy as np
import ml_dtypes

B, C, H, W = 4, 2048, 32, 32
HW = H * W
NH, HD = 8, 256
T = 2 * max(H, W) - 1  # 63
NC = C // 128  # 16 chunks
SX = 16.0      # activation fp8 pre-scale
SW = 1024.0    # weight fp8 pre-scale
SQ = SX * SW   # scale of stored q/k relative to true values

_CACHE = {}


def _hostprep(Wq, bq, Wk, bk, Wv, bv, Wo, bo, rel_emb):
    bf = ml_dtypes.bfloat16
    f8 = ml_dtypes.float8_e4m3
    f32 = np.float32
    Wq, Wk, Wv, Wo = (np.asarray(a, f32) for a in (Wq, Wk, Wv, Wo))
    rel = np.asarray(rel_emb, f32)  # [63, 256]
    ii = np.arange(32)

    # lhsT blobs [co, p, ci*128+m]: one contiguous [128, 2048] DMA per co chunk
    def lchunks(Wm):
        return np.ascontiguousarray(
            Wm.reshape(NC, 128, NC, 128).transpose(2, 1, 0, 3).reshape(NC, 128, C))

    # V weights per head-pair [n2, p, ci*512 + h2*256 + m] (moving operand)
    def rchunks(Wm):
        return np.ascontiguousarray(
            Wm.reshape(NC, 128, NH // 2, 2, HD).transpose(2, 1, 0, 3, 4)
            .reshape(NH // 2, 128, NC * 2 * HD))

    def swpack(blob):  # [NC,128,C] -> DoubleRowSwInterleave layout per cj pair
        b = blob.reshape(NC, 128, NC // 2, 2, 128)  # [co,p,cj,ab,m]
        out = np.empty_like(b)
        out[:, :, :, 0, :] = b[:, :, :, 0, ::-1]
        out[:, :, :, 1, :] = b[:, :, :, 1, ::-1]
        # interleave per logical column: stored col 2j = A[127-j], 2j+1 = B[127-j]
        return np.ascontiguousarray(
            out.transpose(0, 1, 2, 4, 3).reshape(NC, 128, C))

    wq8 = swpack(lchunks(Wq * SW)).astype(f8)
    wk8 = swpack(lchunks(Wk * SW)).astype(f8)
    wob = lchunks(Wo).astype(bf)
    wvb = rchunks(Wv).astype(bf)

    bq_c = np.ascontiguousarray((np.asarray(bq, f32) * SQ).reshape(NC, 128).T)
    bo2 = np.asarray(bv, f32) @ Wo + np.asarray(bo, f32)
    bo2_c = np.ascontiguousarray(bo2.reshape(NC, 128).T)  # [128,16]

    w_idx, k_idx = np.meshgrid(np.arange(32), np.arange(32), indexing="ij")
    # rel_w fold table, w-major [2, 128, 1024], scaled to stored-k units:
    # rwf[ch, p, w*32+k] = SQ * rel[(k-w)%63, ch*128+p]
    rwf = rel[(k_idx - w_idx) % T].reshape(HW, HD) * SQ  # [(w,k), 256]
    rwf = np.ascontiguousarray(rwf.T.reshape(2, 128, HW)).astype(bf)
    # two-period rel_emb.T for QAUG, scaled by SQ so QAUG = q_s * relx is at
    # SQ^2 like q_s*k_s: relx[p, ch*126+u] = SQ * rel[u%63, ch*128+p]
    relx = np.empty((128, 2 * 2 * T), f32)
    for ch in range(2):
        blk = rel[np.arange(2 * T) % T, ch * 128:(ch + 1) * 128]  # [126,128]
        relx[:, ch * 2 * T:(ch + 1) * 2 * T] = blk.T * SQ
    relx = relx.astype(bf)
    # key-side aug channels [96, 1024] w-major: rows 0:63 one-hot rel gather
    # (kaug[t, w*32+k] = t==k), row 63 zero, rows 64:96 block-diag mask
    # (kmask[w', w*32+k] = 0 if w==w' else -1e30). Query side: rows 0:63 QAUG,
    # row 63 zero, rows 64:96 w-indicator.
    kaug = np.zeros((96, HW), f32)
    kaug[k_idx.reshape(-1), np.arange(HW)] = 1.0
    kaug[64:96] = -1e30
    wind = np.zeros((32, HW), f32)
    for w in range(32):
        wind[w, w * 32 + ii] = 1.0  # query col w*32+i
        kaug[64 + w, w * 32 + ii] = 0.0  # key col w*32+k
    kaug = kaug.astype(bf)
    wind = wind.astype(bf)

    return dict(wq8=wq8, wk8=wk8, wob=wob, wvb=wvb, bq_c=bq_c, bo2_c=bo2_c,
                rwf=rwf, relx=relx, kaug=kaug, wind=wind)


def _build(timing_twin=False, loop=1):
    import concourse.bacc as bacc
    import concourse.mybir as mybir
    import concourse.tile as tile

    F32, BF16 = mybir.dt.float32, mybir.dt.bfloat16
    F8 = mybir.dt.float8e4
    DR = mybir.MatmulPerfMode.DoubleRow
    DRS = mybir.MatmulPerfMode.DoubleRowSwInterleave
    nc = bacc.Bacc(None, target_bir_lowering=False)

    if timing_twin:
        # timing-equivalent NEFF: big tensors live in internal DRAM scratch
        # (no per-call host staging), only a tiny external in/out pair.
        def declare(name, shape, dt, isOutput=False):
            return nc.dram_tensor(name, shape, dt)
        tiny_in = nc.declare_dram_parameter("tiny_in", [1, 4], F32, isOutput=False)
        tiny_out = nc.declare_dram_parameter("tiny_out", [1, 4], F32, isOutput=True)
    else:
        declare = nc.declare_dram_parameter

    xq = declare("xq", [C, HW], F8, isOutput=False)
    xr8 = declare("xr8", [C, HW], F8, isOutput=False)
    xrb = declare("xrb", [C, HW], BF16, isOutput=False)
    wq8 = declare("wq8", [NC, 128, C], F8, isOutput=False)
    wk8 = declare("wk8", [NC, 128, C], F8, isOutput=False)
    wob = declare("wob", [NC, 128, C], BF16, isOutput=False)
    wvb = declare("wvb", [NH // 2, 128, NC * 2 * HD], BF16, isOutput=False)
    bq_c = declare("bq_c", [128, NC], F32, isOutput=False)
    bo2_c = declare("bo2_c", [128, NC], F32, isOutput=False)
    rwf = declare("rwf", [2, 128, HW], BF16, isOutput=False)
    relx = declare("relx", [128, 2 * 2 * T], BF16, isOutput=False)
    kaug = declare("kaug", [96, HW], BF16, isOutput=False)
    wind = declare("wind", [32, HW], BF16, isOutput=False)
    out = declare("out", [C, HW], F32, isOutput=True)

    EXP = mybir.ActivationFunctionType.Exp
    ESCALE = 1.0 / (16.0 * SQ * SQ)

    with tile.TileContext(nc) as tc:
        with (
            tc.tile_pool(name="feat", bufs=2) as feat_pool,
            tc.tile_pool(name="featb", bufs=2) as featb_pool,
            tc.tile_pool(name="const", bufs=1) as const_pool,
            tc.tile_pool(name="head", bufs=3) as head_pool,
            tc.tile_pool(name="vpair", bufs=2) as vpair_pool,
            tc.tile_pool(name="wstr8", bufs=3) as wstr8_pool,
            tc.tile_pool(name="wstrb", bufs=3) as wstrb_pool,
            tc.tile_pool(name="probs", bufs=2) as probs_pool,
            tc.tile_pool(name="outs", bufs=2) as outs_pool,
            tc.tile_pool(name="psum", bufs=5, space="PSUM") as psum_pool,
            tc.tile_pool(name="psumb", bufs=1, space="PSUM") as psumb_pool,
            tc.tile_pool(name="psumav", bufs=1, space="PSUM") as psumav_pool,
            tc.tile_pool(name="psumq", bufs=1, space="PSUM") as psumq_pool,
        ):
            # ---- load features + constants (resident) ----
            # xq first (gates the very first Q-proj groups), then xr8/xrb,
            # then constants so the PE cold-start wait is minimal.
            xqt = feat_pool.tile([128, NC * HW], F8, tag="feat8")
            xr8t = feat_pool.tile([128, NC * HW], F8, tag="feat8")
            xrbt = featb_pool.tile([128, NC * HW], BF16, tag="featb")
            attb = featb_pool.tile([128, NC * HW], BF16, tag="featb")
            for cc in range(NC):
                nc.sync.dma_start(xqt[:, cc * HW:(cc + 1) * HW], xq[cc * 128:(cc + 1) * 128, :])
            for cc in range(NC):
                nc.sync.dma_start(xr8t[:, cc * HW:(cc + 1) * HW], xr8[cc * 128:(cc + 1) * 128, :])
            for cc in range(NC):
                nc.sync.dma_start(xrbt[:, cc * HW:(cc + 1) * HW], xrb[cc * 128:(cc + 1) * 128, :])

            c_kaug = const_pool.tile([96, HW], BF16)
            nc.sync.dma_start(c_kaug[:], kaug[:])
            c_wind = const_pool.tile([32, HW], BF16)
            nc.sync.dma_start(c_wind[:], wind[:])
            c_rwf = const_pool.tile([128, 2 * HW], BF16)
            nc.sync.dma_start(c_rwf[:, 0:HW], rwf[0])
            nc.sync.dma_start(c_rwf[:, HW:2 * HW], rwf[1])
            c_relx = const_pool.tile([128, 2 * 2 * T], BF16)
            nc.sync.dma_start(c_relx[:], relx[:])
            c_bq = const_pool.tile([128, NC], F32)
            nc.sync.dma_start(c_bq[:], bq_c[:])
            c_bo = const_pool.tile([128, NC], F32)
            nc.sync.dma_start(c_bo[:], bo2_c[:])

            x3q = xqt[:, :].rearrange("p (ci hw) -> p ci hw", ci=NC)
            x3r = xr8t[:, :].rearrange("p (ci hw) -> p ci hw", ci=NC)
            x3rb = xrbt[:, :].rearrange("p (ci hw) -> p ci hw", ci=NC)
            a3 = attb[:, :].rearrange("p (cc hw) -> p cc hw", cc=NC)

            psb = psumb_pool.tile([128, 512], F32, tag="sa")
            pav = psumav_pool.tile([128, 512], F32, tag="av")

            def att_block(n, sq, sk, sqa, sv2):
                # ---- attention per w-group (runs one head behind the
                # projections, so the softmax DVE/ACT chain overlaps the next
                # head's projection matmuls instead of stalling the in-order
                # PE queue) ----
                for wg in range(8):
                    sc = psb[:, (wg % 4) * 128:(wg % 4 + 1) * 128]
                    nc.tensor.matmul(sc[:], sq[:, wg * 128:(wg + 1) * 128],
                                     sk[:, wg * 128:(wg + 1) * 128],
                                     start=True, stop=False)
                    nc.tensor.matmul(sc[:], sq[:, HW + wg * 128: HW + (wg + 1) * 128],
                                     sk[:, HW + wg * 128: HW + (wg + 1) * 128],
                                     start=False, stop=False)
                    nc.tensor.matmul(sc[:], sqa[:, wg * 128:(wg + 1) * 128],
                                     c_kaug[:, wg * 128:(wg + 1) * 128],
                                     start=False, stop=True)
                    probs = probs_pool.tile([128, 128], BF16, tag="pr")
                    sums = probs_pool.tile([128, 1], F32, tag="sm")
                    recip = probs_pool.tile([128, 1], F32, tag="rc")
                    nc.scalar.activation(probs[:], sc[:], EXP, scale=ESCALE,
                                         accum_out=sums[:])
                    nc.vector.reciprocal(recip[:], sums[:])
                    nc.any.tensor_scalar_mul(probs[:], probs[:], recip[:])
                    probsT = probs_pool.tile([128, 128], BF16, tag="prT")
                    nc.vector.transpose(probsT[:], probs[:])
                    av = pav[:, (wg % 2) * 256:(wg % 2 + 1) * 256]
                    for ch in range(2):
                        svbase = wg * 2 * HD + (n % 2) * HD + ch * 128
                        nc.tensor.matmul(
                            av[ :, ch * 128:(ch + 1) * 128],
                            sv2[:, svbase: svbase + 128],
                            probsT[:], start=True, stop=True)
                    nc.any.tensor_copy(
                        a3[:, n * 2:n * 2 + 2, wg * 128:(wg + 1) * 128],
                        av.rearrange("p (ch x) -> p ch x", ch=2))

            for rep in range(loop):
                prev = None
                for n in range(NH):
                    sq = head_pool.tile([128, 2 * HW], BF16, tag="sq")
                    sk = head_pool.tile([128, 2 * HW], BF16, tag="sk")
                    sqa = head_pool.tile([96, HW], BF16, tag="sqa")
                    if n % 2 == 0:
                        # ---- V projection for the head pair (n, n+1), bf16:
                        # x-stationary (x cj chunk [128,128]), Wv moving
                        # [128, 512] covering both heads. ----
                        sv2 = vpair_pool.tile([128, 8 * 2 * HD], BF16, tag="sv2")
                        swv = vpair_pool.tile([128, NC * 2 * HD], BF16, tag="swv")
                        nc.sync.dma_start(swv[:], wvb[n // 2])
                        wv3 = swv[:, :].rearrange("p (ci m) -> p ci m", ci=NC)
                        for wg in range(8):
                            psv = psum_pool.tile([128, 2 * HD], F32, tag="pp")
                            for cj in range(NC):
                                nc.tensor.matmul(
                                    psv[:],
                                    x3rb[:, cj, wg * 128:(wg + 1) * 128],
                                    wv3[:, cj, :],
                                    start=(cj == 0),
                                    stop=(cj == NC - 1))
                            nc.any.tensor_copy(sv2[:, wg * 2 * HD:(wg + 1) * 2 * HD], psv[:])
                    # aug rows 32:64 zero (kaug one-hot rows t>=32 are all
                    # zero, so sqa rows 32:63 never contribute; keep finite);
                    # rows 64:96 w-indicator. head_pool rotates over 3
                    # buffers, and rows 32:96 are never overwritten, so only
                    # the first three heads (one init per buffer) need this.
                    if rep == 0 and n < 3:
                        nc.vector.memset(sqa[32:64, :], 0.0)
                        nc.vector.tensor_copy(sqa[64:96, :], c_wind[:])

                    # ---- Q / K projections: W.T @ x, single-pass scaled fp8
                    # DoubleRow chains into one PSUM ----
                    for which in range(2):  # 0 = Q, 1 = K
                        hsrc = wq8 if which == 0 else wk8
                        x3 = x3q if which == 0 else x3r
                        dst = sq if which == 0 else sk
                        for co2 in range(2):
                            co = n * 2 + co2
                            wt_h = wstr8_pool.tile([128, C], F8, tag="wl8")
                            nc.sync.dma_start(wt_h[:], hsrc[co])
                            w3h = wt_h[:, :].rearrange("p (ci m) -> p ci m", ci=NC)
                            pss = [psum_pool.tile([128, 512], F32, tag="pp",
                                                  name=f"psqk{h2}")
                                   for h2 in range(2)]
                            for cj in range(NC // 2):
                                for h2 in range(2):
                                    nc.tensor.matmul(
                                        pss[h2][:],
                                        wt_h[:, cj * 256:(cj + 1) * 256],
                                        x3[:, 2 * cj:2 * cj + 2, h2 * 512:(h2 + 1) * 512],
                                        start=(cj == 0),
                                        stop=(cj == NC // 2 - 1),
                                        perf_mode=DRS)
                            for h2 in range(2):
                                dpos = dst[:, co2 * HW + h2 * 512: co2 * HW + (h2 + 1) * 512]
                                if which == 0:
                                    nc.any.tensor_scalar_add(dpos, pss[h2][:], c_bq[:, co:co + 1])
                                else:
                                    nc.any.tensor_add(
                                        dpos, pss[h2][:],
                                        c_rwf[:, co2 * HW + h2 * 512: co2 * HW + (h2 + 1) * 512])

                    # ---- QAUG: per query-row i, rolled rel_emb.T contraction.
                    # Only out rows t'<32 matter (kaug one-hot needs t'==k,
                    # k<32), so the stationary is the 32-col slice
                    # relx[:, 63-i : 95-i] (cheap LDWEIGHTS). Four i's run
                    # concurrently via PE column tiling (tile_position
                    # (0, 32j)): out partitions 32j:32j+32 hold i = ig*4+j.
                    pqa = psumq_pool.tile([128, 256], F32, tag="qa")
                    for ig in range(8):
                        for j in range(4):
                            i = ig * 4 + j
                            for ch in range(2):
                                nc.tensor.matmul(
                                    pqa[32 * j:32 * j + 32, ig * 32:(ig + 1) * 32],
                                    c_relx[:, ch * 2 * T + T - i: ch * 2 * T + T + 32 - i],
                                    sq[:, ch * HW + i: (ch + 1) * HW: 32],
                                    start=(ch == 0), stop=(ch == 1),
                                    tile_position=(0, 32 * j))
                    # pqa[32j+t', ig*32+w] = QAUG[t', col(w, ig*4+j)]
                    for j in range(4):
                        nc.any.tensor_copy(
                            sqa[0:32, :].rearrange("p (w ig j) -> p j ig w", ig=8, j=4)[:, j, :, :],
                            pqa[32 * j:32 * j + 32, :].rearrange("p (ig w) -> p ig w", w=32))

                    # run the PREVIOUS head's attention now: its softmax
                    # chain latency hides under this head's projections.
                    if prev is not None:
                        att_block(*prev)
                    prev = (n, sq, sk, sqa, sv2)
                att_block(*prev)

                # ---- output projection, bf16 weight-stationary ----
                for co in range(NC):
                    wt_b = wstrb_pool.tile([128, C], BF16, tag="wlb")
                    nc.sync.dma_start(wt_b[:], wob[co])
                    w3b = wt_b[:, :].rearrange("p (ci m) -> p ci m", ci=NC)
                    pss = [psum_pool.tile([128, 512], F32, tag="pp",
                                          name=f"pso{h2}")
                           for h2 in range(2)]
                    for cj in range(NC):
                        for h2 in range(2):
                            nc.tensor.matmul(
                                pss[h2][:], w3b[:, cj, :],
                                a3[:, cj, h2 * 512:(h2 + 1) * 512],
                                start=(cj == 0),
                                stop=(cj == NC - 1))
                    for h2 in range(2):
                        ot = outs_pool.tile([128, 512], F32, tag="ot")
                        nc.any.tensor_scalar_add(ot[:], pss[h2][:], c_bo[:, co:co + 1])
                        nc.sync.dma_start(
                            out[co * 128:(co + 1) * 128, h2 * 512:(h2 + 1) * 512], ot[:])

                if timing_twin:
                    tt = outs_pool.tile([1, 4], F32, tag="tt")
                    nc.sync.dma_start(tt[:], tiny_in[:])
                    nc.sync.dma_start(tiny_out[:], tt[:])

            if timing_twin:
                tt = outs_pool.tile([1, 4], F32, tag="tt")
                nc.sync.dma_start(tt[:], tiny_in[:])
                nc.sync.dma_start(tiny_out[:], tt[:])

    nc.finalize()
    return nc


def kernel(left_features, right_features, Wq, bq, Wk, bk, Wv, bv, Wo, bo, rel_emb,
           _trace=False):
    from concourse.bass_utils import run_bass_kernel_spmd

    if "nc" not in _CACHE:
        _CACHE["nc"] = _build()
    nc = _CACHE["nc"]

    consts = _hostprep(Wq, bq, Wk, bk, Wv, bv, Wo, bo, rel_emb)
    lf = np.asarray(left_features, np.float32)
    rf = np.asarray(right_features, np.float32)

    f8 = ml_dtypes.float8_e4m3
    bf = ml_dtypes.bfloat16

    def wmajor(x):  # (C, H, W) -> (C, HW) with col = w*32 + i
        return np.ascontiguousarray(x.transpose(0, 2, 1).reshape(C, HW))

    in_maps = []
    for core in range(8):
        d, b = divmod(core, 4)
        qf = lf[b] if d == 0 else rf[b]
        rfb = rf[b] if d == 0 else lf[b]
        m = dict(consts)
        wq_ = wmajor(qf)
        wr_ = wmajor(rfb)
        m["xq"] = (wq_ * SX).astype(f8)
        m["xr8"] = (wr_ * SX).astype(f8)
        m["xrb"] = wr_.astype(bf)
        in_maps.append(m)

    res = run_bass_kernel_spmd(nc, in_maps, list(range(8)), trace=_trace)
    _CACHE["last_result"] = res

    def unperm(o):  # [C, HW w-major] -> (C, H, W)
        return np.ascontiguousarray(o.reshape(C, W, H).transpose(0, 2, 1))

    wr = np.stack([unperm(res.results[b]["out"]) for b in range(4)])
    wl = np.stack([unperm(res.results[4 + b]["out"]) for b in range(4)])
    left_att = np.concatenate([lf, wr], axis=1)
    right_att = np.concatenate([rf, wl], axis=1)
    return (left_att, right_att)


# revision 11
# speedup vs baseline: 1.0925x; 1.0925x over previous
# Trainium2 Bass kernel for CoAttentionModule (axial co-attention, 8 heads).
#
# Sharding: data-parallel over (direction, batch) = 2 x 4 = 8 NeuronCores.
# Core c computes weighted = _coattention(qf, rf)[b].T for its (d, b) pair;
# the host concatenates [features, weighted] per direction.
#
# On-chip layout: the hw axis is w-major everywhere (col = w*32 + i, i = h
# index); the host pre-permutes features and un-permutes the output. This
# makes every matmul stationary operand a contiguous SBUF slice (walrus
# requires single-free-dim weight APs).
#
# Precision plan (rel err ~1.7e-2 vs the 2e-2 gate, numpy-validated):
#   Q/K projections: single-pass fp8 DoubleRow (e4m3 weights AND e4m3
#     activations, both pre-scaled on host: x*16, W*1024 so weight values
#     clear the e4m3 subnormal region). The resulting q/k are stored bf16 at
#     16384x their true scale; the 16384^-2 is folded into the softmax exp
#     scale, and rel-embedding constants are host-scaled by 16384 to match.
#   V / O projections: plain bf16 (exact to ~2e-3), attention output bf16.
# This halves the Q/K matmul count vs hi+lo and keeps V/O at the bf16
# streaming floor, while the error budget stays comfortably under the gate.
#
# Per-core pipeline (fp32 PSUM accumulation everywhere):
#   qT = Wq8.T @ xq8 (+bq*S)      [c_out, hw]  fp8 DR single pass
#   kT = Wk8.T @ xr8  + RWF*S     RWF[c,(w,k)] = rel_emb[(k-w)%63, c]  (rel_w
#                                 folded into keys; bk cancels in softmax)
#   v  = xrb.T @ Wvb              [(w,k), c]   bf16, x-stationary
#   QAUG[t', col(w,i)] = sum_c relx[(t'-i)%63, c] q[c, col]  (only rows
#                                 t'<32 matter: kaug one-hot needs t'==k<32)
#   scores tile (head n, w-group of 4) [128=(w,i), 128=(w,k)]:
#       q.k' + QAUG.KAUG(one-hot) + WIND.KMASK(-1e30 off-diag mask channels)
#   softmax: exp(scale=1/(16*16384^2)) with accum_out row sums -> reciprocal
#   probsT via DVE 32x32 stream transpose (block-diagonal => exact transpose)
#   avT[c,(w,i)] = v.T @ probsT (bf16); outT = Wob.T @ attT + bo'  bf16
#   (bv folded on host: bo' = bv @ Wo + bo; bk dropped: softmax-invariant)
import numpy as np
import ml_dtypes

B, C, H, W = 4, 2048, 32, 32
HW = H * W
NH, HD = 8, 256
T = 2 * max(H, W) - 1  # 63
NC = C // 128  # 16 chunks
SX = 16.0      # activation fp8 pre-scale
SW = 1024.0    # weight fp8 pre-scale
SQ = SX * SW   # scale of stored q/k relative to true values

_CACHE = {}


def _hostprep(Wq, bq, Wk, bk, Wv, bv, Wo, bo, rel_emb):
    bf = ml_dtypes.bfloat16
    f8 = ml_dtypes.float8_e4m3
    f32 = np.float32
    Wq, Wk, Wv, Wo = (np.asarray(a, f32) for a in (Wq, Wk, Wv, Wo))
    rel = np.asarray(rel_emb, f32)  # [63, 256]
    ii = np.arange(32)

    # lhsT blobs [co, p, ci*128+m]: one contiguous [128, 2048] DMA per co chunk
    def lchunks(Wm):
        return np.ascontiguousarray(
            Wm.reshape(NC, 128, NC, 128).transpose(2, 1, 0, 3).reshape(NC, 128, C))

    # V weights per head-pair [n2, p, ci*512 + h2*256 + m] (moving operand)
    def rchunks(Wm):
        return np.ascontiguousarray(
            Wm.reshape(NC, 128, NH // 2, 2, HD).transpose(2, 1, 0, 3, 4)
            .reshape(NH // 2, 128, NC * 2 * HD))

    def swpack(blob):  # [NC,128,C] -> DoubleRowSwInterleave layout per cj pair
        b = blob.reshape(NC, 128, NC // 2, 2, 128)  # [co,p,cj,ab,m]
        out = np.empty_like(b)
        out[:, :, :, 0, :] = b[:, :, :, 0, ::-1]
        out[:, :, :, 1, :] = b[:, :, :, 1, ::-1]
        # interleave per logical column: stored col 2j = A[127-j], 2j+1 = B[127-j]
        return np.ascontiguousarray(
            out.transpose(0, 1, 2, 4, 3).reshape(NC, 128, C))

    wq8 = swpack(lchunks(Wq * SW)).astype(f8)
    wk8 = swpack(lchunks(Wk * SW)).astype(f8)
    wob = lchunks(Wo).astype(bf)
    wvb = rchunks(Wv).astype(bf)

    bq_c = np.ascontiguousarray((np.asarray(bq, f32) * SQ).reshape(NC, 128).T)
    bo2 = np.asarray(bv, f32) @ Wo + np.asarray(bo, f32)
    bo2_c = np.ascontiguousarray(bo2.reshape(NC, 128).T)  # [128,16]

    w_idx, k_idx = np.meshgrid(np.arange(32), np.arange(32), indexing="ij")
    # rel_w fold table, w-major [2, 128, 1024], scaled to stored-k units:
    # rwf[ch, p, w*32+k] = SQ * rel[(k-w)%63, ch*128+p]
    rwf = rel[(k_idx - w_idx) % T].reshape(HW, HD) * SQ  # [(w,k), 256]
    rwf = np.ascontiguousarray(rwf.T.reshape(2, 128, HW)).astype(bf)
    # two-period rel_emb.T for QAUG, scaled by SQ so QAUG = q_s * relx is at
    # SQ^2 like q_s*k_s: relx[p, ch*126+u] = SQ * rel[u%63, ch*128+p]
    relx = np.empty((128, 2 * 2 * T), f32)
    for ch in range(2):
        blk = rel[np.arange(2 * T) % T, ch * 128:(ch + 1) * 128]  # [126,128]
        relx[:, ch * 2 * T:(ch + 1) * 2 * T] = blk.T * SQ
    relx = relx.astype(bf)
    # key-side aug channels [96, 1024] w-major: rows 0:63 one-hot rel gather
    # (kaug[t, w*32+k] = t==k), row 63 zero, rows 64:96 block-diag mask
    # (kmask[w', w*32+k] = 0 if w==w' else -1e30). Query side: rows 0:63 QAUG,
    # row 63 zero, rows 64:96 w-indicator.
    kaug = np.zeros((96, HW), f32)
    kaug[k_idx.reshape(-1), np.arange(HW)] = 1.0
    kaug[64:96] = -1e30
    wind = np.zeros((32, HW), f32)
    for w in range(32):
        wind[w, w * 32 + ii] = 1.0  # query col w*32+i
        kaug[64 + w, w * 32 + ii] = 0.0  # key col w*32+k
    kaug = kaug.astype(bf)
    wind = wind.astype(bf)

    return dict(wq8=wq8, wk8=wk8, wob=wob, wvb=wvb, bq_c=bq_c, bo2_c=bo2_c,
                rwf=rwf, relx=relx, kaug=kaug, wind=wind)


def _build(timing_twin=False, loop=1):
    import concourse.bacc as bacc
    import concourse.mybir as mybir
    import concourse.tile as tile

    F32, BF16 = mybir.dt.float32, mybir.dt.bfloat16
    F8 = mybir.dt.float8e4
    DR = mybir.MatmulPerfMode.DoubleRow
    DRS = mybir.MatmulPerfMode.DoubleRowSwInterleave
    nc = bacc.Bacc(None, target_bir_lowering=False)

    if timing_twin:
        # timing-equivalent NEFF: big tensors live in internal DRAM scratch
        # (no per-call host staging), only a tiny external in/out pair.
        def declare(name, shape, dt, isOutput=False):
            return nc.dram_tensor(name, shape, dt)
        tiny_in = nc.declare_dram_parameter("tiny_in", [1, 4], F32, isOutput=False)
        tiny_out = nc.declare_dram_parameter("tiny_out", [1, 4], F32, isOutput=True)
    else:
        declare = nc.declare_dram_parameter

    xq = declare("xq", [C, HW], F8, isOutput=False)
    xr8 = declare("xr8", [C, HW], F8, isOutput=False)
    xrb = declare("xrb", [C, HW], BF16, isOutput=False)
    wq8 = declare("wq8", [NC, 128, C], F8, isOutput=False)
    wk8 = declare("wk8", [NC, 128, C], F8, isOutput=False)
    wob = declare("wob", [NC, 128, C], BF16, isOutput=False)
    wvb = declare("wvb", [NH // 2, 128, NC * 2 * HD], BF16, isOutput=False)
    bq_c = declare("bq_c", [128, NC], F32, isOutput=False)
    bo2_c = declare("bo2_c", [128, NC], F32, isOutput=False)
    rwf = declare("rwf", [2, 128, HW], BF16, isOutput=False)
    relx = declare("relx", [128, 2 * 2 * T], BF16, isOutput=False)
    kaug = declare("kaug", [96, HW], BF16, isOutput=False)
    wind = declare("wind", [32, HW], BF16, isOutput=False)
    out = declare("out", [C, HW], F32, isOutput=True)

    EXP = mybir.ActivationFunctionType.Exp
    ESCALE = 1.0 / (16.0 * SQ * SQ)

    with tile.TileContext(nc) as tc:
        with (
            tc.tile_pool(name="feat", bufs=2) as feat_pool,
            tc.tile_pool(name="featb", bufs=2) as featb_pool,
            tc.tile_pool(name="const", bufs=1) as const_pool,
            tc.tile_pool(name="head", bufs=3) as head_pool,
            tc.tile_pool(name="vpair", bufs=2) as vpair_pool,
            tc.tile_pool(name="wstr8", bufs=3) as wstr8_pool,
            tc.tile_pool(name="wstrb", bufs=3) as wstrb_pool,
            tc.tile_pool(name="probs", bufs=2) as probs_pool,
            tc.tile_pool(name="outs", bufs=2) as outs_pool,
            tc.tile_pool(name="psum", bufs=4, space="PSUM") as psum_pool,
            tc.tile_pool(name="psumb", bufs=2, space="PSUM") as psumb_pool,
            tc.tile_pool(name="psumav", bufs=1, space="PSUM") as psumav_pool,
            tc.tile_pool(name="psumq", bufs=1, space="PSUM") as psumq_pool,
        ):
            # ---- load features + constants (resident) ----
            # xq first (gates the very first Q-proj groups), then xr8/xrb,
            # then constants so the PE cold-start wait is minimal.
            xqt = feat_pool.tile([128, NC * HW], F8, tag="feat8")
            xr8t = feat_pool.tile([128, NC * HW], F8, tag="feat8")
            xrbt = featb_pool.tile([128, NC * HW], BF16, tag="featb")
            attb = featb_pool.tile([128, NC * HW], BF16, tag="featb")
            for cc in range(NC):
                nc.sync.dma_start(xqt[:, cc * HW:(cc + 1) * HW], xq[cc * 128:(cc + 1) * 128, :])
            for cc in range(NC):
                nc.sync.dma_start(xr8t[:, cc * HW:(cc + 1) * HW], xr8[cc * 128:(cc + 1) * 128, :])
            for cc in range(NC):
                nc.sync.dma_start(xrbt[:, cc * HW:(cc + 1) * HW], xrb[cc * 128:(cc + 1) * 128, :])

            c_kaug = const_pool.tile([96, HW], BF16)
            nc.sync.dma_start(c_kaug[:], kaug[:])
            c_wind = const_pool.tile([32, HW], BF16)
            nc.sync.dma_start(c_wind[:], wind[:])
            c_rwf = const_pool.tile([128, 2 * HW], BF16)
            nc.sync.dma_start(c_rwf[:, 0:HW], rwf[0])
            nc.sync.dma_start(c_rwf[:, HW:2 * HW], rwf[1])
            c_relx = const_pool.tile([128, 2 * 2 * T], BF16)
            nc.sync.dma_start(c_relx[:], relx[:])
            c_bq = const_pool.tile([128, NC], F32)
            nc.sync.dma_start(c_bq[:], bq_c[:])
            c_bo = const_pool.tile([128, NC], F32)
            nc.sync.dma_start(c_bo[:], bo2_c[:])

            x3q = xqt[:, :].rearrange("p (ci hw) -> p ci hw", ci=NC)
            x3r = xr8t[:, :].rearrange("p (ci hw) -> p ci hw", ci=NC)
            x3rb = xrbt[:, :].rearrange("p (ci hw) -> p ci hw", ci=NC)
            a3 = attb[:, :].rearrange("p (cc hw) -> p cc hw", cc=NC)

            pav = psumav_pool.tile([128, 512], F32, tag="av")

            def att_block(n, sq, sk, sqa, sv2):
                # ---- attention per w-group (runs one head behind the
                # projections, so the softmax DVE/ACT chain overlaps the next
                # head's projection matmuls instead of stalling the in-order
                # PE queue) ----
                for wg in range(8):
                    sct = psumb_pool.tile([128, 128], F32, tag="sa", name="sct")
                    sc = sct[:]
                    nc.tensor.matmul(sc[:], sq[:, wg * 128:(wg + 1) * 128],
                                     sk[:, wg * 128:(wg + 1) * 128],
                                     start=True, stop=False)
                    nc.tensor.matmul(sc[:], sq[:, HW + wg * 128: HW + (wg + 1) * 128],
                                     sk[:, HW + wg * 128: HW + (wg + 1) * 128],
                                     start=False, stop=False)
                    nc.tensor.matmul(sc[:], sqa[:, wg * 128:(wg + 1) * 128],
                                     c_kaug[:, wg * 128:(wg + 1) * 128],
                                     start=False, stop=True)
                    probs = probs_pool.tile([128, 128], BF16, tag="pr")
                    sums = probs_pool.tile([128, 1], F32, tag="sm")
                    recip = probs_pool.tile([128, 1], F32, tag="rc")
                    nc.scalar.activation(probs[:], sc[:], EXP, scale=ESCALE,
                                         accum_out=sums[:])
                    nc.vector.reciprocal(recip[:], sums[:])
                    nc.any.tensor_scalar_mul(probs[:], probs[:], recip[:])
                    probsT = probs_pool.tile([128, 128], BF16, tag="prT")
                    nc.vector.transpose(probsT[:], probs[:])
                    av = pav[:, (wg % 2) * 256:(wg % 2 + 1) * 256]
                    for ch in range(2):
                        svbase = wg * 2 * HD + (n % 2) * HD + ch * 128
                        nc.tensor.matmul(
                            av[ :, ch * 128:(ch + 1) * 128],
                            sv2[:, svbase: svbase + 128],
                            probsT[:], start=True, stop=True)
                    nc.any.tensor_copy(
                        a3[:, n * 2:n * 2 + 2, wg * 128:(wg + 1) * 128],
                        av.rearrange("p (ch x) -> p ch x", ch=2))

            for rep in range(loop):
                prev = None
                for n in range(NH):
                    sq = head_pool.tile([128, 2 * HW], BF16, tag="sq")
                    sk = head_pool.tile([128, 2 * HW], BF16, tag="sk")
                    sqa = head_pool.tile([96, HW], BF16, tag="sqa")
                    if n % 2 == 0:
                        # ---- V projection for the head pair (n, n+1), bf16:
                        # x-stationary (x cj chunk [128,128]), Wv moving
                        # [128, 512] covering both heads. ----
                        sv2 = vpair_pool.tile([128, 8 * 2 * HD], BF16, tag="sv2")
                        swv = vpair_pool.tile([128, NC * 2 * HD], BF16, tag="swv")
                        nc.sync.dma_start(swv[:], wvb[n // 2])
                        wv3 = swv[:, :].rearrange("p (ci m) -> p ci m", ci=NC)
                        for wg in range(8):
                            psv = psum_pool.tile([128, 2 * HD], F32, tag="pp")
                            for cj in range(NC):
                                nc.tensor.matmul(
                                    psv[:],
                                    x3rb[:, cj, wg * 128:(wg + 1) * 128],
                                    wv3[:, cj, :],
                                    start=(cj == 0),
                                    stop=(cj == NC - 1))
                            nc.any.tensor_copy(sv2[:, wg * 2 * HD:(wg + 1) * 2 * HD], psv[:])
                    # aug rows 32:64 zero (kaug one-hot rows t>=32 are all
                    # zero, so sqa rows 32:63 never contribute; keep finite);
                    # rows 64:96 w-indicator. head_pool rotates over 3
                    # buffers, and rows 32:96 are never overwritten, so only
                    # the first three heads (one init per buffer) need this.
                    if rep == 0 and n < 3:
                        nc.vector.memset(sqa[32:64, :], 0.0)
                        nc.vector.tensor_copy(sqa[64:96, :], c_wind[:])

                    # ---- Q / K projections: W.T @ x, single-pass scaled fp8
                    # DoubleRow chains into one PSUM ----
                    for which in range(2):  # 0 = Q, 1 = K
                        hsrc = wq8 if which == 0 else wk8
                        x3 = x3q if which == 0 else x3r
                        dst = sq if which == 0 else sk
                        for co2 in range(2):
                            co = n * 2 + co2
                            wt_h = wstr8_pool.tile([128, C], F8, tag="wl8")
                            nc.sync.dma_start(wt_h[:], hsrc[co])
                            w3h = wt_h[:, :].rearrange("p (ci m) -> p ci m", ci=NC)
                            pss = [psum_pool.tile([128, 512], F32, tag="pp",
                                                  name=f"psqk{h2}")
                                   for h2 in range(2)]
                            for cj in range(NC // 2):
                                for h2 in range(2):
                                    nc.tensor.matmul(
                                        pss[h2][:],
                                        wt_h[:, cj * 256:(cj + 1) * 256],
                                        x3[:, 2 * cj:2 * cj + 2, h2 * 512:(h2 + 1) * 512],
                                        start=(cj == 0),
                                        stop=(cj == NC // 2 - 1),
                                        perf_mode=DRS)
                            for h2 in range(2):
                                dpos = dst[:, co2 * HW + h2 * 512: co2 * HW + (h2 + 1) * 512]
                                if which == 0:
                                    nc.any.tensor_scalar_add(dpos, pss[h2][:], c_bq[:, co:co + 1])
                                else:
                                    nc.any.tensor_add(
                                        dpos, pss[h2][:],
                                        c_rwf[:, co2 * HW + h2 * 512: co2 * HW + (h2 + 1) * 512])

                    # ---- QAUG: per query-row i, rolled rel_emb.T contraction.
                    # Only out rows t'<32 matter (kaug one-hot needs t'==k,
                    # k<32), so the stationary is the 32-col slice
                    # relx[:, 63-i : 95-i] (cheap LDWEIGHTS). Four i's run
                    # concurrently via PE column tiling (tile_position
                    # (0, 32j)): out partitions 32j:32j+32 hold i = ig*4+j.
                    pqa = psumq_pool.tile([128, 256], F32, tag="qa")
                    for ig in range(8):
                        for j in range(4):
                            i = ig * 4 + j
                            for ch in range(2):
                                nc.tensor.matmul(
                                    pqa[32 * j:32 * j + 32, ig * 32:(ig + 1) * 32],
                                    c_relx[:, ch * 2 * T + T - i: ch * 2 * T + T + 32 - i],
                                    sq[:, ch * HW + i: (ch + 1) * HW: 32],
                                    start=(ch == 0), stop=(ch == 1),
                                    tile_position=(0, 32 * j))
                    # pqa[32j+t', ig*32+w] = QAUG[t', col(w, ig*4+j)]
                    for j in range(4):
                        nc.any.tensor_copy(
                            sqa[0:32, :].rearrange("p (w ig j) -> p j ig w", ig=8, j=4)[:, j, :, :],
                            pqa[32 * j:32 * j + 32, :].rearrange("p (ig w) -> p ig w", w=32))

                    # run the PREVIOUS head's attention now: its softmax
                    # chain latency hides under this head's projections.
                    if prev is not None:
                        att_block(*prev)
                    prev = (n, sq, sk, sqa, sv2)
                att_block(*prev)

                # ---- output projection, bf16 weight-stationary ----
                for co in range(NC):
                    wt_b = wstrb_pool.tile([128, C], BF16, tag="wlb")
                    nc.sync.dma_start(wt_b[:], wob[co])
                    w3b = wt_b[:, :].rearrange("p (ci m) -> p ci m", ci=NC)
                    pss = [psum_pool.tile([128, 512], F32, tag="pp",
                                          name=f"pso{h2}")
                           for h2 in range(2)]
                    for cj in range(NC):
                        for h2 in range(2):
                            nc.tensor.matmul(
                                pss[h2][:], w3b[:, cj, :],
                                a3[:, cj, h2 * 512:(h2 + 1) * 512],
                                start=(cj == 0),
                                stop=(cj == NC - 1))
                    for h2 in range(2):
                        ot = outs_pool.tile([128, 512], F32, tag="ot")
                        nc.any.tensor_scalar_add(ot[:], pss[h2][:], c_bo[:, co:co + 1])
                        nc.sync.dma_start(
                            out[co * 128:(co + 1) * 128, h2 * 512:(h2 + 1) * 512], ot[:])

                if timing_twin:
                    tt = outs_pool.tile([1, 4], F32, tag="tt")
                    nc.sync.dma_start(tt[:], tiny_in[:])
                    nc.sync.dma_start(tiny_out[:], tt[:])

            if timing_twin:
                tt = outs_pool.tile([1, 4], F32, tag="tt")
                nc.sync.dma_start(tt[:], tiny_in[:])
                nc.sync.dma_start(tiny_out[:], tt[:])

    nc.finalize()
    return nc


def kernel(left_features, right_features, Wq, bq, Wk, bk, Wv, bv, Wo, bo, rel_emb,
           _trace=False):
    from concourse.bass_utils import run_bass_kernel_spmd

    if "nc" not in _CACHE:
        _CACHE["nc"] = _build()
    nc = _CACHE["nc"]

    consts = _hostprep(Wq, bq, Wk, bk, Wv, bv, Wo, bo, rel_emb)
    lf = np.asarray(left_features, np.float32)
    rf = np.asarray(right_features, np.float32)

    f8 = ml_dtypes.float8_e4m3
    bf = ml_dtypes.bfloat16

    def wmajor(x):  # (C, H, W) -> (C, HW) with col = w*32 + i
        return np.ascontiguousarray(x.transpose(0, 2, 1).reshape(C, HW))

    in_maps = []
    for core in range(8):
        d, b = divmod(core, 4)
        qf = lf[b] if d == 0 else rf[b]
        rfb = rf[b] if d == 0 else lf[b]
        m = dict(consts)
        wq_ = wmajor(qf)
        wr_ = wmajor(rfb)
        m["xq"] = (wq_ * SX).astype(f8)
        m["xr8"] = (wr_ * SX).astype(f8)
        m["xrb"] = wr_.astype(bf)
        in_maps.append(m)

    res = run_bass_kernel_spmd(nc, in_maps, list(range(8)), trace=_trace)
    _CACHE["last_result"] = res

    def unperm(o):  # [C, HW w-major] -> (C, H, W)
        return np.ascontiguousarray(o.reshape(C, W, H).transpose(0, 2, 1))

    wr = np.stack([unperm(res.results[b]["out"]) for b in range(4)])
    wl = np.stack([unperm(res.results[4 + b]["out"]) for b in range(4)])
    left_att = np.concatenate([lf, wr], axis=1)
    right_att = np.concatenate([rf, wl], axis=1)
    return (left_att, right_att)
